# revision 49
# baseline (speedup 1.0000x reference)
"""Cross-attention kernel for 8 trn2 NeuronCores.

Reference computation (per batch b of 16):
  q = Wq @ x, k = Wk @ y, v = Wv @ y          (1x1 convs as channel matmuls)
  q,k l2-normalized over the SPATIAL axis (per (h,d) row)
  sim = 10 * q^T k per head; attn = softmax_j(sim); o = attn @ v^T
  out = Wo @ o + b

Sharding: data-parallel over batch, 2 batches per core, weights replicated.

v3 design (ACT-bound; optimized against the TimelineSim cost model):
  - exp on ACT is the roofline: 64 x [128,1024] tiles = 66.4us busy/core.
  - S_T (q^T k, d=64 contraction) in fp8e4m3 + MatmulPerfMode.DoubleRow
    (0.5 cycles/row, dst partition base must be 0): q quantized RAW (q~N(0,1)
    fits e4m3), k carries the combined l2 scale sq*sk*2^10 (power-of-two gain
    keeps k8 in range; exp scale becomes 10/1024). DoubleRow pair layout
    [32p, 2pair, n] per head (d = p + 32*pair) via a DRAM-scratch reshuffle.
  - softmax denominator: 64-wide ones block in v^T puts the denominator
    pre-broadcast on PV out rows 64:128; normalize = reciprocal + multiply
    per i-half (DVE, PSUM-sourced).
  - zproj contracts kc1 as two 64-row matmuls (heads 2 and 3 separately) so
    the drain tail only waits on the last head's 64-row matmul.
  - Cross-head S_T handoff: head h emits head h+1's first S_T before its own
    last PV, so ACT never gaps at head boundaries (st ring bufs=2 fits this).
  - PSUM (16KB/partition): st [128,1024]f32 x2 (8K) + acc [128,512]f32 x2
    (4K) + op [128,1024]f32 x1 (4K).
  - Weights packed into 2 DMAs (wqk, wvo); batch-1 x/y loads deferred into
    head (0,0) so the fp8 reshuffle round-trip owns the DMA engine early.
"""

import sys

import numpy as np

if "/opt/trn_rl_repo" not in sys.path:
    sys.path.insert(0, "/opt/trn_rl_repo")

NB = 2        # batches per core
C = 256       # channels
N = 1024      # spatial (32*32)
HEADS = 4
DH = 64
HID = 256
NCORES = 8
MAGIC = 0x5F3759DF  # Quake fast inverse-sqrt seed
KGAIN = 1024.0      # power-of-two gain folded into k8; exp scale = 10/KGAIN

_CACHE = {}
PHASES = []


def _mark(nc, label):
    PHASES.append((int(nc.get_next_instruction_name()[2:]), label))


def _quake_rsqrt(eng, pool, p_ap, out_ap, final_scale):
    """out = rsqrt(p) * final_scale for [128,1] fp32 APs on engine `eng`.

    Quake seed + 2 Newton iterations (rel err ~1e-7), no ACT table needed.
    """
    from concourse import mybir

    i32 = mybir.dt.int32
    alu = mybir.AluOpType
    t = pool.tile([128, 1], mybir.dt.float32, tag="qk_rs_t", bufs=4)
    r = pool.tile([128, 1], mybir.dt.float32, tag="qk_rs_r", bufs=4)
    a = pool.tile([128, 1], mybir.dt.float32, tag="qk_rs_a", bufs=4)
    # seed: r0 = bitcast(MAGIC - (bitcast_i32(p) >> 1))
    eng.tensor_scalar(t.bitcast(i32), p_ap.bitcast(i32), 1, None,
                      alu.logical_shift_right)
    eng.tensor_scalar(r.bitcast(i32), t.bitcast(i32), -1, MAGIC,
                      alu.mult, alu.add)
    # Newton 1: r = r * (1.5 - 0.5 * p * r^2)
    eng.scalar_tensor_tensor(a[:], r[:], r[:, 0:1], p_ap,
                             alu.mult, alu.mult)
    eng.tensor_scalar(a[:], a[:], -0.5, 1.5, alu.mult, alu.add)
    eng.tensor_scalar(t[:], a[:], r[:, 0:1], None, alu.mult)
    # Newton 2 (fold final_scale into the last multiply)
    eng.scalar_tensor_tensor(a[:], t[:], t[:, 0:1], p_ap,
                             alu.mult, alu.mult)
    eng.tensor_scalar(a[:], a[:], -0.5, 1.5, alu.mult, alu.add)
    eng.tensor_scalar(out_ap, a[:], t[:, 0:1], final_scale,
                      alu.mult, alu.mult)


def _build_nc():
    from contextlib import ExitStack

    import concourse.tile as tile
    from concourse import bacc, mybir

    f32 = mybir.dt.float32
    f16 = mybir.dt.float16
    f8 = mybir.dt.float8e4
    alu = mybir.AluOpType
    EXP = mybir.ActivationFunctionType.Exp
    DR = mybir.MatmulPerfMode.DoubleRow

    nc = bacc.Bacc("TRN2", target_bir_lowering=False)

    xin = nc.dram_tensor("x", [NB, C, N], f16, kind="ExternalInput")
    yin = nc.dram_tensor("y", [NB, C, N], f16, kind="ExternalInput")
    wqk = nc.dram_tensor("wqk", [128, 4, HID], f16, kind="ExternalInput")
    wvo = nc.dram_tensor("wvo", [128, 4, HID], f16, kind="ExternalInput")
    bo = nc.dram_tensor("b_out", [2, 128, 1], f32, kind="ExternalInput")
    out = nc.dram_tensor("out", [NB, C, N], f32, kind="ExternalOutput")
    # DRAM scratch for the fp8 DoubleRow pair-layout reshuffle
    q8d = nc.dram_tensor("q8_scratch", [NB, 2, 64, 2 * N], f8, kind="Internal")
    k8d = nc.dram_tensor("k8_scratch", [NB, 2, 64, 2 * N], f8, kind="Internal")

    with tile.TileContext(nc) as tc, ExitStack() as ctx:
        consts = ctx.enter_context(tc.tile_pool(name="consts", bufs=1))
        big = ctx.enter_context(tc.tile_pool(name="big", bufs=2))
        sm = ctx.enter_context(tc.tile_pool(name="sm", bufs=4))
        ps = ctx.enter_context(tc.tile_pool(name="ps", bufs=2, space="PSUM"))

        # ---- weight + input loads ------------------------------------
        wqk_sb = consts.tile([128, 4, HID], f16, tag="wqk")
        wvo_sb = consts.tile([128, 4, HID], f16, tag="wvo")
        b_sb = consts.tile([128, 2, 1], f32, tag="bo")
        # warm the ACT exp table while input DMAs are in flight
        warm = sm.tile([128, 1], f32, tag="warm", bufs=1)
        nc.vector.memset(warm[:], 0.0)
        nc.scalar.activation(out=warm[:], in_=warm[:], func=EXP, scale=1.0)
        xts, yts = [], []
        for nb in range(NB):
            xt = big.tile([128, 2, N], f16, tag="xt", bufs=2)
            yt = big.tile([128, 2, N], f16, tag="yt", bufs=2)
            xts.append(xt)
            yts.append(yt)
        nc.sync.dma_start(out=wqk_sb[:], in_=wqk[:])
        nc.sync.dma_start(out=xts[0][:], in_=xin[0].rearrange("(kc p) n -> p kc n", p=128))
        nc.sync.dma_start(out=yts[0][:], in_=yin[0].rearrange("(kc p) n -> p kc n", p=128))
        nc.sync.dma_start(out=wvo_sb[:], in_=wvo[:])
        nc.sync.dma_start(out=b_sb[:], in_=bo.rearrange("kc p n -> p kc n"))

        def load_b1():
            nc.sync.dma_start(out=yts[1][:], in_=yin[1].rearrange("(kc p) n -> p kc n", p=128))
            nc.sync.dma_start(out=xts[1][:], in_=xin[1].rearrange("(kc p) n -> p kc n", p=128))

        # ---------------------------------------------------------------
        # Startup path for (b0, mc0): heads 0,1 run f16 S_T (no fp8
        # round-trip on the critical path). qn = raw f16 q; kn carries the
        # combined scale 1/(||q||*||k||). PSUM goes through the (still
        # unused) st ring so the acc ring can't serialize the q->k chain.
        def proj_qk_f16(nb, mc, qn, kn):
            # PE warm-up: keep the PE busy from ~t=1us so the p-state is at
            # full clock when the real projections arrive.
            wsrc = big.tile([128, 512], f16, tag="wsrc", bufs=1, name="wsrc")
            nc.gpsimd.memset(wsrc[:], 0.0)
            for i in range(8):
                wp = ps.tile([128, 512], f32, tag="acc", bufs=2, name="wp")
                nc.tensor.matmul(wp[:], wsrc[:, 0:128], wsrc[:],
                                 start=True, stop=True)
            qp = ps.tile([128, N], f32, tag="st", bufs=2, name="qp_s")
            for ih in range(2):
                for kc in range(2):
                    nc.tensor.matmul(
                        qp[:, ih * 512:(ih + 1) * 512],
                        wqk_sb[:, kc, mc * 128:(mc + 1) * 128],
                        xts[nb][:, kc, ih * 512:(ih + 1) * 512],
                        start=(kc == 0), stop=(kc == 1))
            kp = ps.tile([128, N], f32, tag="st", bufs=2, name="kp_s")
            for ih in range(2):
                for kc in range(2):
                    nc.tensor.matmul(
                        kp[:, ih * 512:(ih + 1) * 512],
                        wqk_sb[:, 2 + kc, mc * 128:(mc + 1) * 128],
                        yts[nb][:, kc, ih * 512:(ih + 1) * 512],
                        start=(kc == 0), stop=(kc == 1))
            stq = sm.tile([128, 2, 6], f32, tag="stq", bufs=4, name="stq")
            stk = sm.tile([128, 2, 6], f32, tag="stk", bufs=4, name="stk")
            for ih in range(2):
                nc.vector.bn_stats(out=stq[:, ih, :],
                                   in_=qp[:, ih * 512:(ih + 1) * 512])
            nc.vector.tensor_copy(qn[:], qp[:])
            for ih in range(2):
                nc.vector.bn_stats(out=stk[:, ih, :],
                                   in_=kp[:, ih * 512:(ih + 1) * 512])
            mvq = sm.tile([128, 2], f32, tag="mvq", bufs=4, name="mvq")
            mvk = sm.tile([128, 2], f32, tag="mvk", bufs=4, name="mvk")
            nc.vector.bn_aggr(out=mvq[:], in_=stq[:])
            nc.vector.bn_aggr(out=mvk[:], in_=stk[:])
            pqk = sm.tile([128, 1], f32, tag="pqk", bufs=4, name="pqk")
            uq = sm.tile([128, 1], f32, tag="uq", bufs=4, name="uq")
            nc.vector.scalar_tensor_tensor(uq[:], mvq[:, 0:1], mvq[:, 0:1],
                                           mvq[:, 1:2], alu.mult, alu.add)
            nc.vector.scalar_tensor_tensor(pqk[:], mvk[:, 0:1], mvk[:, 0:1],
                                           mvk[:, 1:2], alu.mult, alu.add)
            nc.vector.tensor_tensor(pqk[:], pqk[:], uq[:], alu.mult)
            sck = sm.tile([128, 1], f32, tag="sck", bufs=4, name="sck")
            _quake_rsqrt(nc.vector, sm, pqk[:], sck[:], 1.0 / float(N))
            for ih in range(2):
                nc.vector.tensor_scalar(kn[:, ih * 512:(ih + 1) * 512],
                                        kp[:, ih * 512:(ih + 1) * 512],
                                        sck[:, 0:1], None, alu.mult)

        def proj_qk_f16b(nb, mc, qn, kn):
            state = {}

            def pmm(w4, srct, key, ih):
                pp = ps.tile([128, 512], f32, tag="acc", bufs=2, name="pp")
                for kc in range(2):
                    nc.tensor.matmul(
                        pp[:],
                        wqk_sb[:, w4 + kc, mc * 128:(mc + 1) * 128],
                        srct[:, kc, ih * 512:(ih + 1) * 512],
                        start=(kc == 0), stop=(kc == 1))
                state.setdefault(key, []).append(pp)

            def qstage():
                pmm(0, xts[nb], "q", 0)
                pmm(0, xts[nb], "q", 1)
                qps = state["q"]
                for ih in range(2):
                    nc.vector.tensor_copy(qn[:, ih * 512:(ih + 1) * 512],
                                          qps[ih][:])
                stq = sm.tile([128, 2, 6], f32, tag="stq", bufs=4, name="stq")
                for ih in range(2):
                    nc.vector.bn_stats(out=stq[:, ih, :],
                                       in_=qn[:, ih * 512:(ih + 1) * 512])
                mvq = sm.tile([128, 2], f32, tag="mvq", bufs=4, name="mvq")
                nc.vector.bn_aggr(out=mvq[:], in_=stq[:])
                uq = sm.tile([128, 1], f32, tag="uq", bufs=4, name="uq")
                nc.vector.scalar_tensor_tensor(uq[:], mvq[:, 0:1], mvq[:, 0:1],
                                               mvq[:, 1:2], alu.mult, alu.add)
                state["uq"] = uq

            def kstage():
                pmm(2, yts[nb], "k", 0)
                pmm(2, yts[nb], "k", 1)
                kps = state["k"]
                knr = big.tile([128, N], f16, tag="knrb", bufs=3, name="knrb")
                for ih in range(2):
                    nc.vector.tensor_copy(knr[:, ih * 512:(ih + 1) * 512],
                                          kps[ih][:])
                stk = sm.tile([128, 2, 6], f32, tag="stk", bufs=4, name="stk")
                for ih in range(2):
                    nc.vector.bn_stats(out=stk[:, ih, :],
                                       in_=knr[:, ih * 512:(ih + 1) * 512])
                mvk = sm.tile([128, 2], f32, tag="mvk", bufs=4, name="mvk")
                nc.vector.bn_aggr(out=mvk[:], in_=stk[:])
                pqk = sm.tile([128, 1], f32, tag="pqk", bufs=4, name="pqk")
                nc.vector.scalar_tensor_tensor(pqk[:], mvk[:, 0:1], mvk[:, 0:1],
                                               mvk[:, 1:2], alu.mult, alu.add)
                nc.vector.tensor_tensor(pqk[:], pqk[:], state["uq"][:],
                                        alu.mult)
                sck = sm.tile([128, 1], f32, tag="sck", bufs=4, name="sck")
                _quake_rsqrt(nc.vector, sm, pqk[:], sck[:], 1.0 / float(N))
                # scale-multiply on Pool (SBUF f16 -> f16)
                for ih in range(2):
                    nc.gpsimd.tensor_scalar(kn[:, ih * 512:(ih + 1) * 512],
                                            knr[:, ih * 512:(ih + 1) * 512],
                                            sck[:, 0:1], None, alu.mult)

            return [qstage, kstage]

        def alloc_v(vts_nb):
            for jc in range(8):
                vt = big.tile([128, 4, 128], f16, tag="vt", bufs=18,
                              name=f"vt{jc}")
                vts_nb.append(vt)

        def proj_v(nb, jcs, vts_nb):
            for jc in jcs:
                vp = ps.tile([128, 512], f32, tag="acc", bufs=2)
                for kc in range(2):
                    nc.tensor.matmul(
                        vp[:, 0:HID],
                        yts[nb][:, kc, jc * 128:(jc + 1) * 128],
                        wvo_sb[:, kc, :],
                        start=(kc == 0), stop=(kc == 1))
                vt = vts_nb[jc]
                nc.vector.tensor_copy(vt[:, :, 0:64],
                                      vp[:, 0:HID].rearrange("p (h d) -> p h d", h=4))
                nc.gpsimd.memset(vt[:, :, 64:128], 1.0)

        # one attention head. Fillers fire between the next S_T emission and
        # the PV emission, so a PV stall (waiting on exp) lets ready filler
        # work run first and the handed-off S_T is never queued behind it.
        # Head h emits head h+1's first S_T before its own last PV.
        def make_head(nb, h, q8r2, k8r2, vts_nb, o_sb, qn=None, kn=None):
            mc, ha = h // 2, h % 2

            if qn is not None:
                def st_mm(jc):
                    st = ps.tile([128, N], f32, tag="st", bufs=2)
                    for ih in range(2):
                        nc.tensor.matmul(
                            st[:, ih * 512:(ih + 1) * 512],
                            kn[64 * ha:64 * (ha + 1), jc * 128:(jc + 1) * 128],
                            qn[64 * ha:64 * (ha + 1), ih * 512:(ih + 1) * 512],
                            start=True, stop=True)
                    return st
                scale = 10.0
            else:
                q8r, k8r = q8r2[mc], k8r2[mc]

                def st_mm(jc):
                    st = ps.tile([128, N], f32, tag="st", bufs=2)
                    for ih in range(2):
                        nc.tensor.matmul(
                            st[:, ih * 512:(ih + 1) * 512],
                            k8r[ha * 32:(ha + 1) * 32, :, jc * 128:(jc + 1) * 128],
                            q8r[ha * 32:(ha + 1) * 32, :, ih * 512:(ih + 1) * 512],
                            start=True, stop=True, perf_mode=DR)
                    return st
                scale = 10.0 / KGAIN

            return {"nb": nb, "h": h, "mc": mc, "ha": ha, "vts": vts_nb,
                    "o_sb": o_sb, "st_mm": st_mm, "first_st": None,
                    "scale": scale}

        def run_head(hc, next_hc, fillers=(), handoff=True):
            h, ha, hp = hc["h"], hc["ha"], hc["mc"]
            _mark(nc, f"head(b{hc['nb']},h{h}) start")
            hr = 64 * ha
            o_sb, vts_nb = hc["o_sb"], hc["vts"]
            fi = iter(fillers)
            op = ps.tile([128, N], f32, tag="op", bufs=1, name="op")
            sts = [hc["first_st"] if hc["first_st"] is not None
                   else hc["st_mm"](0)]
            for jc in range(8):
                et = big.tile([128, N], f16, tag="et", bufs=8, name="et")
                nc.scalar.activation(out=et[:], in_=sts[jc][:], func=EXP,
                                     scale=hc["scale"])
                if jc < 7:
                    sts.append(hc["st_mm"](jc + 1))
                elif next_hc is not None and handoff:
                    next_hc["first_st"] = next_hc["st_mm"](0)
                f = next(fi, None)
                if f is not None:
                    _mark(nc, f"head(b{hc['nb']},h{h}) filler jc{jc}")
                    f()
                    _mark(nc, f"head(b{hc['nb']},h{h}) filler jc{jc} end")
                vt = vts_nb[jc]
                for ih in range(2):
                    nc.tensor.matmul(
                        op[:, ih * 512:(ih + 1) * 512],
                        vt[:, h, :],
                        et[:, ih * 512:(ih + 1) * 512],
                        start=(jc == 0), stop=(jc == 7))
            _mark(nc, f"head(b{hc['nb']},h{h}) norm")
            for ihn in range(2):
                sl = slice(ihn * 512, (ihn + 1) * 512)
                db = big.tile([64, 512], f32, tag="db", bufs=8, name="db")
                nc.vector.reciprocal(db[:], op[64:128, sl])
                nc.vector.tensor_tensor(o_sb[hr:hr + 64, hp, sl],
                                        op[0:64, sl], db[:], alu.mult)
            for f in fi:
                _mark(nc, f"head(b{hc['nb']},h{h}) leftover")
                f()
            _mark(nc, f"head(b{hc['nb']},h{h}) end")

        def zproj(nb, o_sb, mcs=(0, 1), ihs=(0, 1)):
            for mc in mcs:
                for ih in ihs:
                    sl = slice(ih * 512, (ih + 1) * 512)
                    msl = slice(mc * 128, (mc + 1) * 128)
                    zp = ps.tile([128, 512], f32, tag="acc", bufs=2)
                    nc.tensor.matmul(zp[:], wvo_sb[:, 2, msl],
                                     o_sb[:, 0, sl], start=True, stop=False)
                    nc.tensor.matmul(zp[:], wvo_sb[:, 3, msl],
                                     o_sb[:, 1, sl], start=False, stop=True)
                    zs = big.tile([128, 512], f32, tag="zs", bufs=8)
                    nc.vector.tensor_scalar(zs[:], zp[:], b_sb[:, mc, 0:1],
                                            None, alu.add)
                    nc.sync.dma_start(
                        out=out[nb, msl, sl],
                        in_=zs[:])

        def alloc_qk():
            q8r2 = [big.tile([64, 2, N], f8, tag="q8r", bufs=4, name=f"q8r{i}")
                    for i in range(2)]
            k8r2 = [big.tile([64, 2, N], f8, tag="k8r", bufs=4, name=f"k8r{i}")
                    for i in range(2)]
            return q8r2, k8r2

        def alloc_o():
            return big.tile([128, 2, N], f16, tag="osb", bufs=2, name="osb")

        # ---- schedule -------------------------------------------------
        o0 = alloc_o()
        o1 = alloc_o()
        qns, kns = [], []
        for i in range(4):
            qns.append(big.tile([128, N], f16, tag="qn", bufs=4, name=f"qn{i}"))
            kns.append(big.tile([128, N], f16, tag="kn", bufs=4, name=f"kn{i}"))
        vts0, vts1 = [], []
        proj_qk_f16(0, 0, qns[0], kns[0])
        with tc.tile_wait_until(0.0072):
            wsrc2 = big.tile([128, 512], f16, tag="wsrc", bufs=1, name="wsrc2")
            nc.gpsimd.memset(wsrc2[:], 0.0)
            for i in range(9):
                wp2 = ps.tile([128, 512], f32, tag="acc", bufs=2, name="wp2")
                nc.tensor.matmul(wp2[:], wsrc2[:, 0:128], wsrc2[:],
                                 start=True, stop=True)
        with tc.tile_wait_until(0.004):
            load_b1()
        qk01 = proj_qk_f16b(0, 1, qns[1], kns[1])
        qk10 = proj_qk_f16b(1, 0, qns[2], kns[2])
        qk11 = proj_qk_f16b(1, 1, qns[3], kns[3])
        alloc_v(vts0)
        alloc_v(vts1)
        with tc.tile_wait_until(0.013):
            qk01[0](); qk01[1]()
        with tc.tile_wait_until(0.014):
            proj_v(0, range(4), vts0)
        with tc.tile_wait_until(0.019):
            proj_v(0, range(4, 8), vts0)
        with tc.tile_wait_until(0.020):
            qk10[0](); qk10[1]()
        with tc.tile_wait_until(0.033):
            qk11[0](); qk11[1]()
        hcs = [make_head(0, 0, None, None, vts0, o0, qns[0], kns[0]),
               make_head(0, 1, None, None, vts0, o0, qns[0], kns[0]),
               make_head(0, 2, None, None, vts0, o0, qns[1], kns[1]),
               make_head(0, 3, None, None, vts0, o0, qns[1], kns[1]),
               make_head(1, 0, None, None, vts1, o1, qns[2], kns[2]),
               make_head(1, 1, None, None, vts1, o1, qns[2], kns[2]),
               make_head(1, 2, None, None, vts1, o1, qns[3], kns[3]),
               make_head(1, 3, None, None, vts1, o1, qns[3], kns[3])]
        run_head(hcs[0], hcs[1])
        run_head(hcs[1], hcs[2])
        with tc.tile_wait_until(0.030):
            proj_v(1, range(4), vts1)
        with tc.tile_wait_until(0.034):
            proj_v(1, range(4, 8), vts1)
        run_head(hcs[2], hcs[3])
        run_head(hcs[3], hcs[4])
        with tc.tile_wait_until(0.039):
            zproj(0, o0)
        run_head(hcs[4], hcs[5])
        run_head(hcs[5], hcs[6])
        run_head(hcs[6], hcs[7])
        run_head(hcs[7], None)
        zproj(1, o1)

    nc.finalize()
    return nc


def _get_nc():
    if "nc" not in _CACHE:
        _CACHE["nc"] = _build_nc()
    return _CACHE["nc"]


def kernel(x, y, w_qkv, w_out, b_out):
    from concourse.bass_utils import run_bass_kernel_spmd

    nc = _get_nc()

    x = np.asarray(x, dtype=np.float32).reshape(16, C, N).astype(np.float16)
    y = np.asarray(y, dtype=np.float32).reshape(16, C, N).astype(np.float16)
    w_qkv = np.asarray(w_qkv, dtype=np.float32)
    wq_t = np.ascontiguousarray(w_qkv[0:HID].T).astype(np.float16)
    wk_t = np.ascontiguousarray(w_qkv[HID:2 * HID].T).astype(np.float16)
    wv_t = np.ascontiguousarray(w_qkv[2 * HID:3 * HID].T).astype(np.float16)
    wo_t = np.ascontiguousarray(np.asarray(w_out, dtype=np.float32).T).astype(np.float16)
    bo = np.ascontiguousarray(
        np.asarray(b_out, dtype=np.float32).reshape(2, 128, 1))

    def pack2(a, b):
        # [128, 4, 256]: [:, 0:2] = a chunks, [:, 2:4] = b chunks, where
        # [:, w*2+kc, n] = w_t[kc*128 + p, n]
        pk = np.empty((128, 4, HID), dtype=np.float16)
        pk[:, 0:2] = a.reshape(2, 128, HID).transpose(1, 0, 2)
        pk[:, 2:4] = b.reshape(2, 128, HID).transpose(1, 0, 2)
        return pk

    wqk = pack2(wq_t, wk_t)
    wvo = pack2(wv_t, wo_t)

    in_maps = []
    for c in range(NCORES):
        in_maps.append({
            "x": np.ascontiguousarray(x[c * NB:(c + 1) * NB]),
            "y": np.ascontiguousarray(y[c * NB:(c + 1) * NB]),
            "wqk": wqk, "wvo": wvo,
            "b_out": bo,
        })

    res = run_bass_kernel_spmd(nc, in_maps, list(range(NCORES)))
    full = np.concatenate([res.results[i]["out"] for i in range(NCORES)], axis=0)
    return full.reshape(16, C, 32, 32)
